# revision 6
# baseline (speedup 1.0000x reference)
"""Trainium2 Bass kernel for nn_ConeFilter.

out[b, f, t] = noisy[b, f, t] * tot_impedance[f], where tot_impedance is a
257-length filter derived from 3 scalar params (tiny compute, done on host in
float32 with the exact op sequence of the reference).

Sharding: data-parallel over batch — 8 cores x 4 batches each. Per core the
shard [4, 257, 4000] is viewed as [1028, 4000] rows; rows are tiled 128 at a
time onto SBUF partitions, multiplied by a per-partition scalar (the filter
value for that row), and stored back. Memory-bound: ~16.4 MB in + 16.4 MB out
per core.
"""

import numpy as np

import concourse.bacc as bacc
import concourse.mybir as mybir
from concourse.tile import TileContext
from concourse.bass_utils import run_bass_kernel_spmd

N_CORES = 8
B, F, T = 32, 257, 4000
BPC = B // N_CORES            # batches per core
ROWS = BPC * F                # 1028 rows per core
NT = (ROWS + 127) // 128      # 9 row-tiles (8 full + 1 of 4 rows)
PADROWS = NT * 128

SAMPLE_RATE = 44100.0
STEPS = 100
C = np.float32(343.0)
RHO = np.float32(1.293)
PI = np.float32(3.1415927)

# Filled with the BassKernelResults of the most recent run (for test harness).
LAST_RESULT = None

_nc_cache = None


def _compute_filter(x0, angle, depth):
    """Replicates the reference's tot_impedance computation in float32."""
    f32 = np.float32
    x0 = np.asarray(x0, dtype=f32)
    angle = np.asarray(angle, dtype=f32)
    depth = np.asarray(depth, dtype=f32)

    freq_map = np.linspace(0.0, SAMPLE_RATE / 2.0, F, dtype=f32)
    myx0 = np.maximum(x0, f32(0))
    mydepth = np.maximum(depth, f32(0))
    with np.errstate(over="ignore", invalid="ignore", divide="ignore"):
        sig = f32(1) / (f32(1) + np.exp(-angle))
        myangle = (sig + f32(1)) / f32(2) * (PI / f32(2))

        i = np.arange(STEPS, dtype=f32)
        x = myx0 + i * mydepth / f32(STEPS)
        r = np.tan(myangle) * x

        numer = freq_map[None, :] * x[:, None]
        frac = numer / (numer + C)
        impedance = frac * RHO * C / (PI * r * r)[:, None]
        tot = np.prod(impedance, axis=0, dtype=f32)
    return tot.astype(f32)  # [F]


def _build():
    global _nc_cache
    if _nc_cache is not None:
        return _nc_cache
    # Bacc (not plain Bass): its finalize() runs generate_event_semaphores,
    # which splits multi-wait sync_info into EventSemaphore instructions —
    # TRN2 ISA structs have a single wait slot and walrus rejects more.
    nc = bacc.Bacc(None)
    f32 = mybir.dt.float32
    x = nc.declare_dram_parameter("x", [ROWS, T], f32, isOutput=False)
    fcol = nc.declare_dram_parameter("fcol", [128, NT], f32, isOutput=False)
    y = nc.declare_dram_parameter("y", [ROWS, T], f32, isOutput=True)

    with TileContext(nc) as tc:
        with (
            tc.tile_pool(name="fp", bufs=1) as fp,
            tc.tile_pool(name="data", bufs=NT) as pool,
        ):
            ft = fp.tile([128, NT], f32, tag="ft")
            nc.sync.dma_start(out=ft[:], in_=fcol[:])
            # The TensorScalarPtr ISA struct has a single sync-wait slot.
            # Absorb the filter-DMA wait into DVE's vector clock with a tiny
            # read of ft, so the tensor_scalars below only ever wait on their
            # own data load. Fresh slots (bufs=NT) avoid WAR waits entirely.
            sink = fp.tile([128, 1], f32, tag="sink")
            nc.vector.tensor_copy(out=sink[:], in_=ft[:, 0:1])
            for i in range(NT):
                r0 = 128 * i
                p = min(128, ROWS - r0)
                t = pool.tile([128, T], f32)
                nc.sync.dma_start(out=t[:p], in_=x[r0 : r0 + p])
                nc.vector.tensor_scalar_mul(
                    out=t[:p], in0=t[:p], scalar1=ft[:p, i : i + 1]
                )
                nc.sync.dma_start(out=y[r0 : r0 + p], in_=t[:p])
    nc.finalize()
    _nc_cache = nc
    return nc


def kernel(noisy, x0, angle, depth):
    global LAST_RESULT
    noisy = np.ascontiguousarray(np.asarray(noisy, dtype=np.float32))
    assert noisy.shape == (B, F, T), noisy.shape

    tot = _compute_filter(x0, angle, depth)           # [F]
    rows = np.tile(tot, BPC)                          # [ROWS]
    padded = np.zeros(PADROWS, np.float32)
    padded[:ROWS] = rows
    fcol = np.ascontiguousarray(padded.reshape(NT, 128).T)  # [128, NT]

    nc = _build()
    in_maps = []
    for c in range(N_CORES):
        shard = noisy[c * BPC : (c + 1) * BPC].reshape(ROWS, T)
        in_maps.append({"x": shard, "fcol": fcol})

    res = run_bass_kernel_spmd(nc, in_maps, list(range(N_CORES)))
    LAST_RESULT = res

    out = np.empty((B, F, T), np.float32)
    for c in range(N_CORES):
        out[c * BPC : (c + 1) * BPC] = res.results[c]["y"].reshape(BPC, F, T)
    return out


# revision 7
# speedup vs baseline: 1.0982x; 1.0982x over previous
"""Trainium2 Bass kernel for nn_ConeFilter.

out[b, f, t] = noisy[b, f, t] * tot_impedance[f], where tot_impedance is a
257-length filter derived from 3 scalar params (tiny compute, done on host in
float32 with the exact op sequence of the reference).

Sharding: data-parallel over batch — 8 cores x 4 batches each. Per core the
shard [4, 257, 4000] is viewed as [1028, 4000] rows. The first 1024 rows are
processed as 8 [128, 4000] tiles (per-partition scalar multiply, one filter
value per row). The ragged last 4 rows are passed as a separate [128, 125]
view (contiguous memory, 32 partitions per row) so every DMA uses all 128
partitions. Memory-bound: ~16.4 MB in + 16.4 MB out per core.
"""

import numpy as np

import concourse.bacc as bacc
import concourse.mybir as mybir
from concourse.tile import TileContext
from concourse.bass_utils import run_bass_kernel_spmd

N_CORES = 8
B, F, T = 32, 257, 4000
BPC = B // N_CORES            # batches per core
ROWS = BPC * F                # 1028 rows per core
NFULL = ROWS // 128           # 8 full [128, T] tiles
TAILROWS = ROWS - NFULL * 128  # 4 leftover rows
TAILW = TAILROWS * T // 128   # 125 elems/partition in the [128, .] tail view
NT = NFULL + 1

SAMPLE_RATE = 44100.0
STEPS = 100
C = np.float32(343.0)
RHO = np.float32(1.293)
PI = np.float32(3.1415927)

# Filled with the BassKernelResults of the most recent run (for test harness).
LAST_RESULT = None

_nc_cache = None


def _compute_filter(x0, angle, depth):
    """Replicates the reference's tot_impedance computation in float32."""
    f32 = np.float32
    x0 = np.asarray(x0, dtype=f32)
    angle = np.asarray(angle, dtype=f32)
    depth = np.asarray(depth, dtype=f32)

    freq_map = np.linspace(0.0, SAMPLE_RATE / 2.0, F, dtype=f32)
    myx0 = np.maximum(x0, f32(0))
    mydepth = np.maximum(depth, f32(0))
    with np.errstate(over="ignore", invalid="ignore", divide="ignore"):
        sig = f32(1) / (f32(1) + np.exp(-angle))
        myangle = (sig + f32(1)) / f32(2) * (PI / f32(2))

        i = np.arange(STEPS, dtype=f32)
        x = myx0 + i * mydepth / f32(STEPS)
        r = np.tan(myangle) * x

        numer = freq_map[None, :] * x[:, None]
        frac = numer / (numer + C)
        impedance = frac * RHO * C / (PI * r * r)[:, None]
        tot = np.prod(impedance, axis=0, dtype=f32)
    return tot.astype(f32)  # [F]


def _build():
    global _nc_cache
    if _nc_cache is not None:
        return _nc_cache
    # Bacc (not plain Bass): its finalize() runs generate_event_semaphores,
    # which splits multi-wait sync_info into EventSemaphore instructions —
    # TRN2 ISA structs have a single wait slot and walrus rejects more.
    nc = bacc.Bacc(None)
    f32 = mybir.dt.float32
    x = nc.declare_dram_parameter("x", [NFULL * 128, T], f32, isOutput=False)
    xt = nc.declare_dram_parameter("xt", [128, TAILW], f32, isOutput=False)
    fcol = nc.declare_dram_parameter("fcol", [128, NT], f32, isOutput=False)
    y = nc.declare_dram_parameter("y", [NFULL * 128, T], f32, isOutput=True)
    yt = nc.declare_dram_parameter("yt", [128, TAILW], f32, isOutput=True)

    with TileContext(nc) as tc:
        with (
            tc.tile_pool(name="fp", bufs=1) as fp,
            tc.tile_pool(name="data", bufs=NT) as pool,
        ):
            ft = fp.tile([128, NT], f32, tag="ft")
            nc.sync.dma_start(out=ft[:], in_=fcol[:])
            # The TensorScalarPtr ISA struct has a single sync-wait slot.
            # Absorb the filter-DMA wait into DVE's vector clock with a tiny
            # read of ft, so the tensor_scalars below only ever wait on their
            # own data load. Fresh slots (bufs=NT) avoid WAR waits entirely.
            sink = fp.tile([128, 1], f32, tag="sink")
            nc.vector.tensor_copy(out=sink[:], in_=ft[:, 0:1])
            for i in range(NFULL):
                r0 = 128 * i
                t = pool.tile([128, T], f32)
                nc.sync.dma_start(out=t[:], in_=x[r0 : r0 + 128])
                nc.vector.tensor_scalar_mul(
                    out=t[:], in0=t[:], scalar1=ft[:, i : i + 1]
                )
                nc.sync.dma_start(out=y[r0 : r0 + 128], in_=t[:])
            # Tail: [128, TAILW] view of the last TAILROWS rows.
            tt = pool.tile([128, TAILW], f32, tag="tail")
            nc.sync.dma_start(out=tt[:], in_=xt[:])
            nc.vector.tensor_scalar_mul(
                out=tt[:], in0=tt[:], scalar1=ft[:, NFULL : NFULL + 1]
            )
            nc.sync.dma_start(out=yt[:], in_=tt[:])
    nc.finalize()
    _nc_cache = nc
    return nc


def kernel(noisy, x0, angle, depth):
    global LAST_RESULT
    noisy = np.ascontiguousarray(np.asarray(noisy, dtype=np.float32))
    assert noisy.shape == (B, F, T), noisy.shape

    tot = _compute_filter(x0, angle, depth)           # [F]
    rows = np.tile(tot, BPC)                          # [ROWS]
    fcol = np.zeros((128, NT), np.float32)
    for i in range(NFULL):
        fcol[:, i] = rows[i * 128 : (i + 1) * 128]
    # Tail view: partition p holds elements of row (NFULL*128 + p//32).
    fcol[:, NFULL] = np.repeat(rows[NFULL * 128 :], 128 // TAILROWS)
    fcol = np.ascontiguousarray(fcol)

    nc = _build()
    in_maps = []
    for c in range(N_CORES):
        shard = noisy[c * BPC : (c + 1) * BPC].reshape(ROWS, T)
        in_maps.append(
            {
                "x": shard[: NFULL * 128],
                "xt": shard[NFULL * 128 :].reshape(128, TAILW),
                "fcol": fcol,
            }
        )

    res = run_bass_kernel_spmd(nc, in_maps, list(range(N_CORES)))
    LAST_RESULT = res

    out = np.empty((B, F, T), np.float32)
    for c in range(N_CORES):
        oc = out[c * BPC : (c + 1) * BPC].reshape(ROWS, T)
        oc[: NFULL * 128] = res.results[c]["y"]
        oc[NFULL * 128 :] = res.results[c]["yt"].reshape(TAILROWS, T)
    return out
